# revision 13
# baseline (speedup 1.0000x reference)
"""Sliding-window causal GQA attention with ALiBi, head-sharded across 8 TRN2 cores.

Problem: B=2, S=2048, H=32, D=128, KV=8 (GQA group 4), window=(1024,0),
softmax scale 1/sqrt(128), ALiBi slopes = 0.8409...^(h+1).
Sharding: core c owns heads 4c..4c+3 and KV head c. No collectives.

Device kernel (per core), transposed-flash formulation with host-side
transposes:
  The host supplies Q^T ([head*d, token], fp16) and K^T ([d, token], fp16,
  pre-scaled by 1/sqrt(D)), so the device never transposes anything.
  Per (head, batch): one DMA brings in Q^T for all 16 query blocks; per
  query block, S^T[j,q] = matmul(lhsT=K^T block, rhs=Q^T block) per
  kv-block into shared PSUM chunks; DVE adds the ALiBi+mask table; one Exp
  per query block produces P^T (fp16) that feeds the PV matmuls directly.
  V carries an appended ones column per 128-token block, so the softmax
  denominator accumulates into column 128 of the [128,129] PSUM output;
  normalization is a DVE reciprocal + Act per-partition scale, written into
  a per-(head,batch) output strip that leaves in a single DMA.

I/O is fp16; the jitted shard_map executor is built once and cached; the
alibi tables are uploaded to the devices once and reused across calls.
"""

import math
import sys
from contextlib import ExitStack

import numpy as np

sys.path.insert(0, "/opt/trn_rl_repo")

import jax
from jax.sharding import Mesh, NamedSharding, PartitionSpec
from jax.experimental.shard_map import shard_map

import concourse.bass as bass
import concourse.mybir as mybir
import concourse.tile as tile
from concourse import bacc
from concourse.bass2jax import (
    _bass_exec_p,
    install_neuronx_cc_hook,
    partition_id_tensor,
)

B, S = 2, 2048
H, D = 32, 128
KV = 8
WINDOW = 1024
SCALE = 1.0 / math.sqrt(D)
NCORES = 8
HPC = H // NCORES     # heads per core
NQ = S // 128         # 16 query blocks per batch
NDELTA = 9            # kj in [qi-8, qi]
NEG = -1e30
NKBLK = B * S // 128  # 32 kv token blocks

F32 = mybir.dt.float32
F16 = mybir.dt.float16


def _slopes():
    start = 2.0 ** (-(2.0 ** (-(math.log2(H) - 3))))
    return [start * start**i for i in range(H)]


def build_kernel():
    nc = bacc.Bacc("TRN2", target_bir_lowering=False, debug=False)

    qt_d = nc.dram_tensor("qt", [HPC * D, B * S], F16, kind="ExternalInput").ap()
    kt_d = nc.dram_tensor("kt", [D, B * S], F16, kind="ExternalInput").ap()
    v_d = nc.dram_tensor("v", [B * S, D], F16, kind="ExternalInput").ap()
    # per head: columns ordered delta=8..0, each 128 wide; entry[r, c] =
    # exp(-slope*(128*delta + c - r)), 0 outside the causal/window band, so
    # P = exp(S) * table applies ALiBi and the mask multiplicatively.
    a_d = nc.dram_tensor("alibi", [128, HPC * NDELTA * 128], F16, kind="ExternalInput").ap()
    o_d = nc.dram_tensor("out", [B * S, HPC * D], F16, kind="ExternalOutput").ap()

    with tile.TileContext(nc) as tc, ExitStack() as ctx:
        const = ctx.enter_context(tc.tile_pool(name="const", bufs=1))
        qtp = ctx.enter_context(tc.tile_pool(name="qtp", bufs=2))
        tp = ctx.enter_context(tc.tile_pool(name="tp", bufs=4))
        pp = ctx.enter_context(tc.tile_pool(name="pp", bufs=4))
        outp = ctx.enter_context(tc.tile_pool(name="outp", bufs=2))
        dnp = ctx.enter_context(tc.tile_pool(name="dnp", bufs=4))
        ps_s = ctx.enter_context(tc.tile_pool(name="ps_s", bufs=2, space="PSUM"))
        ps_o = ctx.enter_context(tc.tile_pool(name="ps_o", bufs=3, space="PSUM"))

        atab = const.tile([128, HPC * NDELTA * 128], F16)
        nc.sync.dma_start(atab[:], a_d[:, :])

        kt = const.tile([128, B * S], F16)
        nc.sync.dma_start(kt[:], kt_d[:, :])

        # V resident: per 128-token block, 128 d-columns + a ones column.
        vt = const.tile([128, NKBLK * 129], F16)
        nc.vector.memset(vt[:], 1.0)
        v_r = v_d.rearrange("(n p) d -> n p d", p=128)
        for t in range(NKBLK):
            nc.sync.dma_start(vt[:, t * 129 : t * 129 + 128], v_r[t, :, :])

        # [p, n, c] view: partition-major so a [128, NQ*128] SBUF strip maps
        # to one DMA (per-partition: NQ runs of 256B).
        o_r = o_d.rearrange("(n p) hd -> p n hd", p=128)

        for h in range(HPC):
            for b in range(B):
                qta = qtp.tile([128, S], F16, tag="qta")
                nc.sync.dma_start(qta[:], qt_d[h * D : (h + 1) * D, b * S : (b + 1) * S])
                o_all = outp.tile([128, NQ * 128], F16, tag="oall")

                for qi in range(NQ):
                    kj0 = max(0, qi - 8)
                    nkj = qi - kj0 + 1
                    e_sb = tp.tile([128, NDELTA * 128], F16, tag="e")
                    p_sb = pp.tile([128, NDELTA * 128], F16, tag="p")
                    o_ps = ps_o.tile([128, 129], F32, tag="o")
                    qt_blk = qta[:, qi * 128 : (qi + 1) * 128]

                    nchunk = (nkj + 7) // 8
                    for ci in range(nchunk):
                        c0 = kj0 + ci * 8
                        w = min(8, kj0 + nkj - c0)
                        wc = w * 128
                        off = (c0 - kj0) * 128
                        s_ps = ps_s.tile([128, 1024], F32, tag="s")
                        for j in range(w):
                            kj = c0 + j
                            tb = b * NQ + kj
                            nc.tensor.matmul(
                                s_ps[:, j * 128 : (j + 1) * 128],
                                kt[:, tb * 128 : (tb + 1) * 128],
                                qt_blk,
                            )
                        nc.scalar.activation(
                            e_sb[:, off : off + wc],
                            s_ps[:, :wc],
                            mybir.ActivationFunctionType.Exp,
                        )

                    # ALiBi + mask applied multiplicatively (fp16, DVE 2x mode)
                    acol0 = h * NDELTA * 128 + (8 - (qi - kj0)) * 128
                    nc.vector.tensor_tensor(
                        p_sb[:, : nkj * 128],
                        e_sb[:, : nkj * 128],
                        atab[:, acol0 : acol0 + nkj * 128],
                        op=mybir.AluOpType.mult,
                    )

                    for j2 in range(nkj):
                        kj = kj0 + j2
                        tb = b * NQ + kj
                        nc.tensor.matmul(
                            o_ps[:],
                            p_sb[:, j2 * 128 : (j2 + 1) * 128],
                            vt[:, tb * 129 : (tb + 1) * 129],
                            start=(j2 == 0),
                            stop=(j2 == nkj - 1),
                        )

                    drec = dnp.tile([128, 1], F32, tag="drec")
                    nc.vector.reciprocal(drec[:], o_ps[:, 128:129])
                    nc.vector.tensor_scalar_mul(
                        o_all[:, qi * 128 : (qi + 1) * 128], o_ps[:, :128], drec[:]
                    )

                nc.sync.dma_start(
                    o_r[:, b * NQ : (b + 1) * NQ, h * D : (h + 1) * D],
                    o_all[:],
                )
    nc.compile()
    return nc


def _alibi_tables(slopes):
    """[128, HPC*9*128] fp16, transposed ([j-row, q-col] layout): per head,
    column blocks delta=8..0; entry = exp(-slope*dist) with
    dist = 128*delta + c - r, and exactly 0 where dist < 0 (causal) or
    dist > WINDOW (sliding window)."""
    r = np.arange(128)[:, None]
    c = np.arange(128)[None, :]
    cols = []
    for s in slopes:
        for d in range(8, -1, -1):
            dist = 128 * d + c - r
            a = np.where(
                (dist < 0) | (dist > WINDOW), 0.0, np.exp(-s * dist.astype(np.float64))
            )
            cols.append(a)
    return np.concatenate(cols, axis=1).astype(np.float16)


class _State:
    pass


_STATE = None


def _get_state():
    global _STATE
    if _STATE is not None:
        return _STATE
    st = _State()
    st.nc = build_kernel()
    install_neuronx_cc_hook()
    nc = st.nc

    partition_name = nc.partition_id_tensor.name if nc.partition_id_tensor else None
    in_names, out_names, out_avals = [], [], []
    for alloc in nc.m.functions[0].allocations:
        if not isinstance(alloc, mybir.MemoryLocationSet):
            continue
        name = alloc.memorylocations[0].name
        if alloc.kind == "ExternalInput":
            if name != partition_name:
                in_names.append(name)
        elif alloc.kind == "ExternalOutput":
            out_names.append(name)
            out_avals.append(
                jax.core.ShapedArray(tuple(alloc.tensor_shape), mybir.dt.np(alloc.dtype))
            )
    in_names_all = list(in_names)
    if partition_name is not None:
        in_names_all.append(partition_name)

    def _body(*args):
        operands = list(args)
        if partition_name is not None:
            operands.append(partition_id_tensor())
        outs = _bass_exec_p.bind(
            *operands,
            out_avals=tuple(out_avals),
            in_names=tuple(in_names_all),
            out_names=tuple(out_names),
            lowering_input_output_aliases=(),
            sim_require_finite=True,
            sim_require_nnan=True,
            nc=nc,
        )
        return tuple(outs)

    devices = jax.devices()[:NCORES]
    mesh = Mesh(np.asarray(devices), ("core",))
    spec = PartitionSpec("core")
    st.sharded = jax.jit(
        shard_map(
            _body,
            mesh=mesh,
            in_specs=(spec,) * len(in_names),
            out_specs=(spec,) * len(out_names),
            check_rep=False,
        )
    )
    st.in_names = in_names
    st.out_names = out_names

    slopes = _slopes()
    atab_global = np.concatenate(
        [_alibi_tables(slopes[c * HPC : (c + 1) * HPC]) for c in range(NCORES)], axis=0
    )
    st.alibi_dev = jax.device_put(atab_global, NamedSharding(mesh, spec))
    st.alibi_dev.block_until_ready()
    _STATE = st
    return st


def kernel(q, k, v):
    st = _get_state()
    q16 = np.asarray(q).astype(np.float16)
    # [T, NCORES*512] -> per-core transposed [NCORES*512, T]
    qt = np.ascontiguousarray(
        q16.reshape(B * S, NCORES, HPC * D).transpose(1, 2, 0)
    ).reshape(NCORES * HPC * D, B * S)
    k16 = (np.asarray(k) * SCALE).astype(np.float16)
    kt = np.ascontiguousarray(
        k16.reshape(B * S, NCORES, D).transpose(1, 2, 0)
    ).reshape(NCORES * D, B * S)
    v16 = np.asarray(v).astype(np.float16)
    vg = np.ascontiguousarray(
        v16.reshape(B * S, NCORES, D).transpose(1, 0, 2)
    ).reshape(NCORES * B * S, D)
    feeds = {"qt": qt, "kt": kt, "v": vg, "alibi": st.alibi_dev}
    outs = st.sharded(*[feeds[n] for n in st.in_names])
    out = np.asarray(outs[0])  # [NCORES*B*S, HPC*D] fp16
    return np.ascontiguousarray(
        out.reshape(NCORES, B * S, HPC * D).transpose(1, 0, 2).reshape(B * S, H * D)
    ).astype(np.float32)


# revision 20
# speedup vs baseline: 1.0697x; 1.0697x over previous
"""Sliding-window causal GQA attention with ALiBi, head-sharded across 8 TRN2 cores.

Problem: B=2, S=2048, H=32, D=128, KV=8 (GQA group 4), window=(1024,0),
softmax scale 1/sqrt(128), ALiBi slopes = 0.8409...^(h+1).
Sharding: core c owns heads 4c..4c+3 and KV head c. No collectives.

Device kernel (per core), transposed-flash formulation with host-side
transposes:
  The host supplies Q^T ([head*d, token], fp16) and K^T ([d, token], fp16,
  pre-scaled by 1/sqrt(D)), so the device never transposes anything.
  Per (head, batch): one DMA brings in Q^T for all 16 query blocks; per
  query block, S^T[j,q] = matmul(lhsT=K^T block, rhs=Q^T block) per
  kv-block into shared PSUM chunks; DVE adds the ALiBi+mask table; one Exp
  per query block produces P^T (fp16) that feeds the PV matmuls directly.
  V carries an appended ones column per 128-token block, so the softmax
  denominator accumulates into column 128 of the [128,129] PSUM output;
  normalization is a DVE reciprocal + Act per-partition scale, written into
  a per-(head,batch) output strip that leaves in a single DMA.

I/O is fp16; the jitted shard_map executor is built once and cached; the
alibi tables are uploaded to the devices once and reused across calls.
"""

import math
import sys
from contextlib import ExitStack

import numpy as np

sys.path.insert(0, "/opt/trn_rl_repo")

import jax
from jax.sharding import Mesh, NamedSharding, PartitionSpec
from jax.experimental.shard_map import shard_map

import concourse.bass as bass
import concourse.mybir as mybir
import concourse.tile as tile
from concourse import bacc
from concourse.bass2jax import (
    _bass_exec_p,
    install_neuronx_cc_hook,
    partition_id_tensor,
)

B, S = 2, 2048
H, D = 32, 128
KV = 8
WINDOW = 1024
SCALE = 1.0 / math.sqrt(D)
NCORES = 8
HPC = H // NCORES     # heads per core
NQ = S // 128         # 16 query blocks per batch
NDELTA = 9            # kj in [qi-8, qi]
NEG = -1e30
NKBLK = B * S // 128  # 32 kv token blocks

F32 = mybir.dt.float32
F16 = mybir.dt.float16


def _slopes():
    start = 2.0 ** (-(2.0 ** (-(math.log2(H) - 3))))
    return [start * start**i for i in range(H)]


def build_kernel():
    nc = bacc.Bacc("TRN2", target_bir_lowering=False, debug=False)

    qt_d = nc.dram_tensor("qt", [HPC * D, B * S], F16, kind="ExternalInput").ap()
    kt_d = nc.dram_tensor("kt", [D, B * S], F16, kind="ExternalInput").ap()
    v_d = nc.dram_tensor("v", [B * S, D], F16, kind="ExternalInput").ap()
    # per head: columns ordered delta=8..0, each 128 wide; entry[r, c] =
    # exp(-slope*(128*delta + c - r)), 0 outside the causal/window band, so
    # P = exp(S) * table applies ALiBi and the mask multiplicatively.
    a_d = nc.dram_tensor("alibi", [128, HPC * NDELTA * 128], F16, kind="ExternalInput").ap()
    o_d = nc.dram_tensor("out", [B * S, HPC * D], F16, kind="ExternalOutput").ap()

    with tile.TileContext(nc) as tc, ExitStack() as ctx:
        const = ctx.enter_context(tc.tile_pool(name="const", bufs=1))
        qtp = ctx.enter_context(tc.tile_pool(name="qtp", bufs=5))
        tp = ctx.enter_context(tc.tile_pool(name="tp", bufs=4))
        pp = ctx.enter_context(tc.tile_pool(name="pp", bufs=4))
        outp = ctx.enter_context(tc.tile_pool(name="outp", bufs=5))
        dnp = ctx.enter_context(tc.tile_pool(name="dnp", bufs=4))
        ps_s = ctx.enter_context(tc.tile_pool(name="ps_s", bufs=4, space="PSUM"))
        ps_o = ctx.enter_context(tc.tile_pool(name="ps_o", bufs=3, space="PSUM"))

        atab = const.tile([128, HPC * NDELTA * 128], F16)
        nc.sync.dma_start(atab[:], a_d[:, :])

        kt = const.tile([128, B * S], F16)
        nc.sync.dma_start(kt[:], kt_d[:, :])

        # V resident: per 128-token block, 128 d-columns + a ones column.
        vt = const.tile([128, NKBLK * 129], F16)
        nc.vector.memset(vt[:], 1.0)
        v_r = v_d.rearrange("(n p) d -> n p d", p=128)
        for t in range(NKBLK):
            nc.sync.dma_start(vt[:, t * 129 : t * 129 + 128], v_r[t, :, :])

        # [p, n, c] view: partition-major so a [128, NQ*128] SBUF strip maps
        # to one DMA (per-partition: NQ runs of 256B).
        o_r = o_d.rearrange("(n p) hd -> p n hd", p=128)

        for b in range(B):
            qtas, o_alls = [], []
            for h in range(HPC):
                qta = qtp.tile([128, S], F16, tag="qta")
                nc.sync.dma_start(qta[:], qt_d[h * D : (h + 1) * D, b * S : (b + 1) * S])
                qtas.append(qta)
                o_alls.append(outp.tile([128, NQ * 128], F16, tag="oall", name="o_all"))

            for qi in range(NQ):
                for h in range(HPC):
                    qta = qtas[h]
                    o_all = o_alls[h]
                    kj0 = max(0, qi - 8)
                    nkj = qi - kj0 + 1
                    e_sb = tp.tile([128, NDELTA * 128], F16, tag="e")
                    p_sb = pp.tile([128, NDELTA * 128], F16, tag="p")
                    o_ps = ps_o.tile([128, 129], F32, tag="o")
                    qt_blk = qta[:, qi * 128 : (qi + 1) * 128]

                    nchunk = (nkj + 3) // 4
                    for ci in range(nchunk):
                        c0 = kj0 + ci * 4
                        w = min(4, kj0 + nkj - c0)
                        wc = w * 128
                        off = (c0 - kj0) * 128
                        s_ps = ps_s.tile([128, 512], F32, tag="s")
                        for j in range(w):
                            kj = c0 + j
                            tb = b * NQ + kj
                            nc.tensor.matmul(
                                s_ps[:, j * 128 : (j + 1) * 128],
                                kt[:, tb * 128 : (tb + 1) * 128],
                                qt_blk,
                            )
                        nc.scalar.activation(
                            e_sb[:, off : off + wc],
                            s_ps[:, :wc],
                            mybir.ActivationFunctionType.Exp,
                        )

                    # ALiBi + mask applied multiplicatively (fp16, DVE 2x mode)
                    acol0 = h * NDELTA * 128 + (8 - (qi - kj0)) * 128
                    nc.vector.tensor_tensor(
                        p_sb[:, : nkj * 128],
                        e_sb[:, : nkj * 128],
                        atab[:, acol0 : acol0 + nkj * 128],
                        op=mybir.AluOpType.mult,
                    )

                    for j2 in range(nkj):
                        kj = kj0 + j2
                        tb = b * NQ + kj
                        nc.tensor.matmul(
                            o_ps[:],
                            p_sb[:, j2 * 128 : (j2 + 1) * 128],
                            vt[:, tb * 129 : (tb + 1) * 129],
                            start=(j2 == 0),
                            stop=(j2 == nkj - 1),
                        )

                    drec = dnp.tile([128, 1], F32, tag="drec")
                    nc.vector.reciprocal(drec[:], o_ps[:, 128:129])
                    nc.vector.tensor_scalar_mul(
                        o_all[:, qi * 128 : (qi + 1) * 128], o_ps[:, :128], drec[:]
                    )

            for h in range(HPC):
                nc.sync.dma_start(
                    o_r[:, b * NQ : (b + 1) * NQ, h * D : (h + 1) * D],
                    o_alls[h][:],
                )
    nc.compile()
    return nc


def _alibi_tables(slopes):
    """[128, HPC*9*128] fp16, transposed ([j-row, q-col] layout): per head,
    column blocks delta=8..0; entry = exp(-slope*dist) with
    dist = 128*delta + c - r, and exactly 0 where dist < 0 (causal) or
    dist > WINDOW (sliding window)."""
    r = np.arange(128)[:, None]
    c = np.arange(128)[None, :]
    cols = []
    for s in slopes:
        for d in range(8, -1, -1):
            dist = 128 * d + c - r
            a = np.where(
                (dist < 0) | (dist > WINDOW), 0.0, np.exp(-s * dist.astype(np.float64))
            )
            cols.append(a)
    return np.concatenate(cols, axis=1).astype(np.float16)


class _State:
    pass


_STATE = None


def _get_state():
    global _STATE
    if _STATE is not None:
        return _STATE
    st = _State()
    st.nc = build_kernel()
    install_neuronx_cc_hook()
    nc = st.nc

    partition_name = nc.partition_id_tensor.name if nc.partition_id_tensor else None
    in_names, out_names, out_avals = [], [], []
    for alloc in nc.m.functions[0].allocations:
        if not isinstance(alloc, mybir.MemoryLocationSet):
            continue
        name = alloc.memorylocations[0].name
        if alloc.kind == "ExternalInput":
            if name != partition_name:
                in_names.append(name)
        elif alloc.kind == "ExternalOutput":
            out_names.append(name)
            out_avals.append(
                jax.core.ShapedArray(tuple(alloc.tensor_shape), mybir.dt.np(alloc.dtype))
            )
    in_names_all = list(in_names)
    if partition_name is not None:
        in_names_all.append(partition_name)

    def _body(*args):
        operands = list(args)
        if partition_name is not None:
            operands.append(partition_id_tensor())
        outs = _bass_exec_p.bind(
            *operands,
            out_avals=tuple(out_avals),
            in_names=tuple(in_names_all),
            out_names=tuple(out_names),
            lowering_input_output_aliases=(),
            sim_require_finite=True,
            sim_require_nnan=True,
            nc=nc,
        )
        return tuple(outs)

    devices = jax.devices()[:NCORES]
    mesh = Mesh(np.asarray(devices), ("core",))
    spec = PartitionSpec("core")
    st.sharded = jax.jit(
        shard_map(
            _body,
            mesh=mesh,
            in_specs=(spec,) * len(in_names),
            out_specs=(spec,) * len(out_names),
            check_rep=False,
        )
    )
    st.in_names = in_names
    st.out_names = out_names

    slopes = _slopes()
    atab_global = np.concatenate(
        [_alibi_tables(slopes[c * HPC : (c + 1) * HPC]) for c in range(NCORES)], axis=0
    )
    st.alibi_dev = jax.device_put(atab_global, NamedSharding(mesh, spec))
    st.alibi_dev.block_until_ready()
    _STATE = st
    return st


def kernel(q, k, v):
    st = _get_state()
    q16 = np.asarray(q).astype(np.float16)
    # [T, NCORES*512] -> per-core transposed [NCORES*512, T]
    qt = np.ascontiguousarray(
        q16.reshape(B * S, NCORES, HPC * D).transpose(1, 2, 0)
    ).reshape(NCORES * HPC * D, B * S)
    k16 = (np.asarray(k) * SCALE).astype(np.float16)
    kt = np.ascontiguousarray(
        k16.reshape(B * S, NCORES, D).transpose(1, 2, 0)
    ).reshape(NCORES * D, B * S)
    v16 = np.asarray(v).astype(np.float16)
    vg = np.ascontiguousarray(
        v16.reshape(B * S, NCORES, D).transpose(1, 0, 2)
    ).reshape(NCORES * B * S, D)
    feeds = {"qt": qt, "kt": kt, "v": vg, "alibi": st.alibi_dev}
    outs = st.sharded(*[feeds[n] for n in st.in_names])
    out = np.asarray(outs[0])  # [NCORES*B*S, HPC*D] fp16
    return np.ascontiguousarray(
        out.reshape(NCORES, B * S, HPC * D).transpose(1, 0, 2).reshape(B * S, H * D)
    ).astype(np.float32)


# revision 23
# speedup vs baseline: 1.1122x; 1.0397x over previous
"""Sliding-window causal GQA attention with ALiBi, head-sharded across 8 TRN2 cores.

Problem: B=2, S=2048, H=32, D=128, KV=8 (GQA group 4), window=(1024,0),
softmax scale 1/sqrt(128), ALiBi slopes = 0.8409...^(h+1).
Sharding: core c owns heads 4c..4c+3 and KV head c. No collectives.

Device kernel (per core), transposed-flash formulation with host-side
transposes:
  The host supplies Q^T ([head*d, token], fp16) and K^T ([d, token], fp16,
  pre-scaled by 1/sqrt(D)), so the device never transposes anything.
  Per (head, batch): one DMA brings in Q^T for all 16 query blocks; per
  query block, S^T[j,q] = matmul(lhsT=K^T block, rhs=Q^T block) per
  kv-block into shared PSUM chunks; DVE adds the ALiBi+mask table; one Exp
  per query block produces P^T (fp16) that feeds the PV matmuls directly.
  V carries an appended ones column per 128-token block, so the softmax
  denominator accumulates into column 128 of the [128,129] PSUM output;
  normalization is a DVE reciprocal + Act per-partition scale, written into
  a per-(head,batch) output strip that leaves in a single DMA.

I/O is fp16; the jitted shard_map executor is built once and cached; the
alibi tables are uploaded to the devices once and reused across calls.
"""

import math
import sys
from contextlib import ExitStack

import numpy as np

sys.path.insert(0, "/opt/trn_rl_repo")

import jax
from jax.sharding import Mesh, NamedSharding, PartitionSpec
from jax.experimental.shard_map import shard_map

import concourse.bass as bass
import concourse.mybir as mybir
import concourse.tile as tile
from concourse import bacc
from concourse.bass2jax import (
    _bass_exec_p,
    install_neuronx_cc_hook,
    partition_id_tensor,
)

B, S = 2, 2048
H, D = 32, 128
KV = 8
WINDOW = 1024
SCALE = 1.0 / math.sqrt(D)
NCORES = 8
HPC = H // NCORES     # heads per core
NQ = S // 128         # 16 query blocks per batch
NDELTA = 9            # kj in [qi-8, qi]
NEG = -1e30
NKBLK = B * S // 128  # 32 kv token blocks

F32 = mybir.dt.float32
F16 = mybir.dt.float16


def _slopes():
    start = 2.0 ** (-(2.0 ** (-(math.log2(H) - 3))))
    return [start * start**i for i in range(H)]


def build_kernel():
    nc = bacc.Bacc("TRN2", target_bir_lowering=False, debug=False)

    qt_d = nc.dram_tensor("qt", [HPC * D, B * S], F16, kind="ExternalInput").ap()
    kt_d = nc.dram_tensor("kt", [D, B * S], F16, kind="ExternalInput").ap()
    v_d = nc.dram_tensor("v", [B * S, D], F16, kind="ExternalInput").ap()
    # per head: columns ordered delta=8..0, each 128 wide; entry[r, c] =
    # exp(-slope*(128*delta + c - r)), 0 outside the causal/window band, so
    # P = exp(S) * table applies ALiBi and the mask multiplicatively.
    a_d = nc.dram_tensor("alibi", [128, HPC * NDELTA * 128], F16, kind="ExternalInput").ap()
    o_d = nc.dram_tensor("out", [B * S, HPC * D], F16, kind="ExternalOutput").ap()

    with tile.TileContext(nc) as tc, ExitStack() as ctx:
        const = ctx.enter_context(tc.tile_pool(name="const", bufs=1))
        qtp = ctx.enter_context(tc.tile_pool(name="qtp", bufs=2))
        tp = ctx.enter_context(tc.tile_pool(name="tp", bufs=3))
        pp = ctx.enter_context(tc.tile_pool(name="pp", bufs=3))
        outp = ctx.enter_context(tc.tile_pool(name="outp", bufs=2))
        dnp = ctx.enter_context(tc.tile_pool(name="dnp", bufs=4))
        ps_s = ctx.enter_context(tc.tile_pool(name="ps_s", bufs=4, space="PSUM"))
        ps_o = ctx.enter_context(tc.tile_pool(name="ps_o", bufs=3, space="PSUM"))

        atab = const.tile([128, HPC * NDELTA * 128], F16)
        nc.sync.dma_start(atab[:], a_d[:, :])

        kt = const.tile([128, B * S], F16)
        nc.sync.dma_start(kt[:], kt_d[:, :])

        # V resident: per 128-token block, 128 d-columns + a ones column.
        vt = const.tile([128, NKBLK * 129], F16)
        nc.vector.memset(vt[:], 1.0)
        v_r = v_d.rearrange("(n p) d -> n p d", p=128)
        for t in range(NKBLK):
            nc.sync.dma_start(vt[:, t * 129 : t * 129 + 128], v_r[t, :, :])

        # [p, n, c] view: partition-major so a [128, NQ*128] SBUF strip maps
        # to one DMA (per-partition: NQ runs of 256B).
        o_r = o_d.rearrange("(n p) hd -> p n hd", p=128)

        for h in range(HPC):
            for b in range(B):
                qta = qtp.tile([128, S], F16, tag="qta")
                nc.sync.dma_start(qta[:], qt_d[h * D : (h + 1) * D, b * S : (b + 1) * S])
                o_all = outp.tile([128, NQ * 128], F16, tag="oall")

                for qi in range(NQ):
                    kj0 = max(0, qi - 8)
                    nkj = qi - kj0 + 1
                    e_sb = tp.tile([128, NDELTA * 128], F16, tag="e")
                    p_sb = pp.tile([128, NDELTA * 128], F16, tag="p")
                    o_ps = ps_o.tile([128, 129], F32, tag="o")
                    qt_blk = qta[:, qi * 128 : (qi + 1) * 128]

                    nchunk = (nkj + 3) // 4
                    for ci in range(nchunk):
                        c0 = kj0 + ci * 4
                        w = min(4, kj0 + nkj - c0)
                        wc = w * 128
                        off = (c0 - kj0) * 128
                        s_ps = ps_s.tile([128, 512], F32, tag="s")
                        for j in range(w):
                            kj = c0 + j
                            tb = b * NQ + kj
                            nc.tensor.matmul(
                                s_ps[:, j * 128 : (j + 1) * 128],
                                kt[:, tb * 128 : (tb + 1) * 128],
                                qt_blk,
                            )
                        nc.scalar.activation(
                            e_sb[:, off : off + wc],
                            s_ps[:, :wc],
                            mybir.ActivationFunctionType.Exp,
                        )

                    # ALiBi + mask applied multiplicatively (fp16, DVE 2x mode)
                    acol0 = h * NDELTA * 128 + (8 - (qi - kj0)) * 128
                    nc.vector.tensor_tensor(
                        p_sb[:, : nkj * 128],
                        e_sb[:, : nkj * 128],
                        atab[:, acol0 : acol0 + nkj * 128],
                        op=mybir.AluOpType.mult,
                    )

                    for j2 in range(nkj):
                        kj = kj0 + j2
                        tb = b * NQ + kj
                        nc.tensor.matmul(
                            o_ps[:],
                            p_sb[:, j2 * 128 : (j2 + 1) * 128],
                            vt[:, tb * 129 : (tb + 1) * 129],
                            start=(j2 == 0),
                            stop=(j2 == nkj - 1),
                        )

                    drec = dnp.tile([128, 1], F32, tag="drec")
                    nc.vector.reciprocal(drec[:], o_ps[:, 128:129])
                    nc.vector.tensor_scalar_mul(
                        o_all[:, qi * 128 : (qi + 1) * 128], o_ps[:, :128], drec[:]
                    )

                nc.sync.dma_start(
                    o_r[:, b * NQ : (b + 1) * NQ, h * D : (h + 1) * D],
                    o_all[:],
                )
    nc.compile()
    return nc


def _alibi_tables(slopes):
    """[128, HPC*9*128] fp16, transposed ([j-row, q-col] layout): per head,
    column blocks delta=8..0; entry = exp(-slope*dist) with
    dist = 128*delta + c - r, and exactly 0 where dist < 0 (causal) or
    dist > WINDOW (sliding window)."""
    r = np.arange(128)[:, None]
    c = np.arange(128)[None, :]
    cols = []
    for s in slopes:
        for d in range(8, -1, -1):
            dist = 128 * d + c - r
            a = np.where(
                (dist < 0) | (dist > WINDOW), 0.0, np.exp(-s * dist.astype(np.float64))
            )
            cols.append(a)
    return np.concatenate(cols, axis=1).astype(np.float16)


class _State:
    pass


_STATE = None


def _get_state():
    global _STATE
    if _STATE is not None:
        return _STATE
    st = _State()
    st.nc = build_kernel()
    install_neuronx_cc_hook()
    nc = st.nc

    partition_name = nc.partition_id_tensor.name if nc.partition_id_tensor else None
    in_names, out_names, out_avals = [], [], []
    for alloc in nc.m.functions[0].allocations:
        if not isinstance(alloc, mybir.MemoryLocationSet):
            continue
        name = alloc.memorylocations[0].name
        if alloc.kind == "ExternalInput":
            if name != partition_name:
                in_names.append(name)
        elif alloc.kind == "ExternalOutput":
            out_names.append(name)
            out_avals.append(
                jax.core.ShapedArray(tuple(alloc.tensor_shape), mybir.dt.np(alloc.dtype))
            )
    in_names_all = list(in_names)
    if partition_name is not None:
        in_names_all.append(partition_name)

    def _body(*args):
        operands = list(args)
        if partition_name is not None:
            operands.append(partition_id_tensor())
        outs = _bass_exec_p.bind(
            *operands,
            out_avals=tuple(out_avals),
            in_names=tuple(in_names_all),
            out_names=tuple(out_names),
            lowering_input_output_aliases=(),
            sim_require_finite=True,
            sim_require_nnan=True,
            nc=nc,
        )
        return tuple(outs)

    devices = jax.devices()[:NCORES]
    mesh = Mesh(np.asarray(devices), ("core",))
    spec = PartitionSpec("core")
    st.sharded = jax.jit(
        shard_map(
            _body,
            mesh=mesh,
            in_specs=(spec,) * len(in_names),
            out_specs=(spec,) * len(out_names),
            check_rep=False,
        )
    )
    st.in_names = in_names
    st.out_names = out_names

    slopes = _slopes()
    atab_global = np.concatenate(
        [_alibi_tables(slopes[c * HPC : (c + 1) * HPC]) for c in range(NCORES)], axis=0
    )
    st.alibi_dev = jax.device_put(atab_global, NamedSharding(mesh, spec))
    st.alibi_dev.block_until_ready()
    _STATE = st
    return st


def kernel(q, k, v):
    st = _get_state()
    q16 = np.asarray(q).astype(np.float16)
    # [T, NCORES*512] -> per-core transposed [NCORES*512, T]
    qt = np.ascontiguousarray(
        q16.reshape(B * S, NCORES, HPC * D).transpose(1, 2, 0)
    ).reshape(NCORES * HPC * D, B * S)
    k16 = (np.asarray(k) * SCALE).astype(np.float16)
    kt = np.ascontiguousarray(
        k16.reshape(B * S, NCORES, D).transpose(1, 2, 0)
    ).reshape(NCORES * D, B * S)
    v16 = np.asarray(v).astype(np.float16)
    vg = np.ascontiguousarray(
        v16.reshape(B * S, NCORES, D).transpose(1, 0, 2)
    ).reshape(NCORES * B * S, D)
    feeds = {"qt": qt, "kt": kt, "v": vg, "alibi": st.alibi_dev}
    outs = st.sharded(*[feeds[n] for n in st.in_names])
    out = np.asarray(outs[0])  # [NCORES*B*S, HPC*D] fp16
    return np.ascontiguousarray(
        out.reshape(NCORES, B * S, HPC * D).transpose(1, 0, 2).reshape(B * S, H * D)
    ).astype(np.float32)
